# revision 2
# baseline (speedup 1.0000x reference)
"""Trainium2 Bass kernel for the DependencyParser biaffine arc scorer.

scores[b,i,j] = W2 @ tanh(Wa@X[b,i] + Wb@X[b,j] + b1) + b2

Shapes (hardcoded): X [32, 96, 512], W1 [512, 1024], b1 [512],
W2 [1, 512], b2 [1].  Output [32, 96, 96] fp32.

Sharding: data-parallel over batch B=32 -> 4 batches per core x 8 cores,
weights replicated.
"""

import numpy as np
import ml_dtypes

B, N, H = 32, 96, 512
NCORES = 8
BPC = B // NCORES          # batches per core
P = 128                    # partitions
NKC = H // P               # 4 k-chunks
NHC = H // P               # 4 h-chunks
IB = 16                    # i-block size
NIB = N // IB              # 6 i-blocks per batch
FBLK = IB * N              # 1536 free elems per (kc, iblock)
NG = FBLK // 384           # 4 matmul groups of 384 cols

_CACHE = {}


def _build():
    """Build + compile the per-core Bass module (same program on all cores)."""
    import concourse.bass as bass
    import concourse.mybir as mybir
    import concourse.tile as tile
    from concourse import bacc

    f32 = mybir.dt.float32
    bf16 = mybir.dt.bfloat16
    Tanh = mybir.ActivationFunctionType.Tanh

    nc = bacc.Bacc("TRN2", target_bir_lowering=False, debug=False)

    xt_d = nc.dram_tensor("xt", [BPC, H, N], f32, kind="ExternalInput")
    wat_d = nc.dram_tensor("wat", [H, H], f32, kind="ExternalInput")
    wbt_d = nc.dram_tensor("wbt", [H, H], f32, kind="ExternalInput")
    b1_d = nc.dram_tensor("b1c", [P, NKC], f32, kind="ExternalInput")
    w2_d = nc.dram_tensor("w2c", [P, NKC], bf16, kind="ExternalInput")
    sc_d = nc.dram_tensor("scores", [BPC, N * N], f32, kind="ExternalOutput")

    with tile.TileContext(nc) as tc:
        with (
            tc.tile_pool(name="const", bufs=1) as cpool,
            tc.tile_pool(name="xt", bufs=2) as xpool,
            tc.tile_pool(name="hab", bufs=2) as habpool,
            tc.tile_pool(name="tpre", bufs=6) as tprepool,
            tc.tile_pool(name="ttan", bufs=2) as ttanpool,
            tc.tile_pool(name="scout", bufs=3) as scpool,
            tc.tile_pool(name="psum_h", bufs=2, space="PSUM") as psum_h,
            tc.tile_pool(name="psum_s", bufs=3, space="PSUM") as psum_sp,
        ):
            # ---- constants ----
            wat_s = cpool.tile([P, NHC * H], f32, tag="wat")
            wbt_s = cpool.tile([P, NHC * H], f32, tag="wbt")
            for hc in range(NHC):
                nc.sync.dma_start(
                    wat_s[:, hc * H:(hc + 1) * H], wat_d[hc * P:(hc + 1) * P, :]
                )
                nc.sync.dma_start(
                    wbt_s[:, hc * H:(hc + 1) * H], wbt_d[hc * P:(hc + 1) * P, :]
                )
            b1_s = cpool.tile([P, NKC], f32, tag="b1")
            nc.sync.dma_start(b1_s[:], b1_d[:])
            w2_s = cpool.tile([P, NKC], bf16, tag="w2")
            nc.sync.dma_start(w2_s[:], w2_d[:])

            for b in range(BPC):
                # ---- load X_b^T [512, 96] as 4 h-chunks ----
                xt_s = xpool.tile([P, NHC * N], f32)
                for hc in range(NHC):
                    nc.sync.dma_start(
                        xt_s[:, hc * N:(hc + 1) * N],
                        xt_d[b, hc * P:(hc + 1) * P, :],
                    )

                # ---- Ha/Hb = Wa@X^T, Wb@X^T  (fp32, k on partitions) ----
                ps_ha = psum_h.tile([P, NKC * N], f32, tag="ha")
                ps_hb = psum_h.tile([P, NKC * N], f32, tag="hb")
                for kc in range(NKC):
                    for hc in range(NHC):
                        nc.tensor.matmul(
                            ps_ha[:, kc * N:(kc + 1) * N],
                            wat_s[:, hc * H + kc * P: hc * H + (kc + 1) * P],
                            xt_s[:, hc * N:(hc + 1) * N],
                            start=(hc == 0),
                            stop=(hc == NHC - 1),
                        )
                for kc in range(NKC):
                    for hc in range(NHC):
                        nc.tensor.matmul(
                            ps_hb[:, kc * N:(kc + 1) * N],
                            wbt_s[:, hc * H + kc * P: hc * H + (kc + 1) * P],
                            xt_s[:, hc * N:(hc + 1) * N],
                            start=(hc == 0),
                            stop=(hc == NHC - 1),
                        )

                # ---- move to SBUF; fold b1 into Ha (stays fp32) ----
                ha_s = habpool.tile([P, NKC * N], f32, tag="ha_s")
                for kc in range(NKC):
                    nc.vector.tensor_scalar_add(
                        ha_s[:, kc * N:(kc + 1) * N],
                        ps_ha[:, kc * N:(kc + 1) * N],
                        b1_s[:, kc:kc + 1],
                    )
                hb_s = habpool.tile([P, NKC * N], bf16, tag="hb_s")
                nc.vector.tensor_copy(hb_s[:], ps_hb[:])

                # ---- main loop: broadcast-add + tanh + W2 contraction ----
                for ib in range(NIB):
                    ttan = ttanpool.tile([P, NKC * FBLK], bf16)
                    for kc in range(NKC):
                        tpre = tprepool.tile([P, FBLK], bf16)
                        for i in range(IB):
                            ii = ib * IB + i
                            nc.vector.tensor_scalar_add(
                                tpre[:, i * N:(i + 1) * N],
                                hb_s[:, kc * N:(kc + 1) * N],
                                ha_s[:, kc * N + ii: kc * N + ii + 1],
                            )
                        nc.scalar.activation(
                            ttan[:, kc * FBLK:(kc + 1) * FBLK], tpre[:], Tanh
                        )
                    ps_s = psum_sp.tile([P, 384], f32)
                    for g in range(NG):
                        for kc in range(NKC):
                            nc.tensor.matmul(
                                ps_s[32 * g:32 * g + 1, :],
                                w2_s[:, kc:kc + 1],
                                ttan[:, kc * FBLK + g * 384: kc * FBLK + (g + 1) * 384],
                                start=(kc == 0),
                                stop=(kc == NKC - 1),
                                tile_position=(0, 32 * g),
                            )
                    sc_s = scpool.tile([P, 384], f32)
                    nc.vector.tensor_copy(sc_s[:], ps_s[:])
                    sc_view = sc_s[:].rearrange("(g r) f -> g r f", r=32)[:, 0, :]
                    nc.sync.dma_start(
                        sc_d[b, ib * FBLK:(ib + 1) * FBLK].rearrange(
                            "(g f) -> g f", g=NG
                        ),
                        sc_view,
                    )

    nc.compile()
    return nc


def _get_nc():
    if "nc" not in _CACHE:
        _CACHE["nc"] = _build()
    return _CACHE["nc"]


def kernel(encoded_sequence, W1, b1, W2, b2):
    from concourse import bass_utils

    nc = _get_nc()

    x = np.asarray(encoded_sequence, dtype=np.float32)
    W1 = np.asarray(W1, dtype=np.float32)
    b1 = np.asarray(b1, dtype=np.float32)
    W2 = np.asarray(W2, dtype=np.float32)
    b2 = np.asarray(b2, dtype=np.float32)

    wat = np.ascontiguousarray(W1[:, :H].T)           # [h, k]
    wbt = np.ascontiguousarray(W1[:, H:].T)           # [h, k]
    b1c = np.ascontiguousarray(b1.reshape(NKC, P).T)  # [128, 4]
    w2c = np.ascontiguousarray(W2[0].reshape(NKC, P).T).astype(ml_dtypes.bfloat16)
    xt = np.ascontiguousarray(x.transpose(0, 2, 1))   # [B, h, n]

    in_maps = []
    for c in range(NCORES):
        in_maps.append({
            "xt": np.ascontiguousarray(xt[c * BPC:(c + 1) * BPC]),
            "wat": wat,
            "wbt": wbt,
            "b1c": b1c,
            "w2c": w2c,
        })

    res = bass_utils.run_bass_kernel_spmd(nc, in_maps, core_ids=list(range(NCORES)))
    out = np.concatenate(
        [res.results[c]["scores"].reshape(BPC, N, N) for c in range(NCORES)], axis=0
    )
    return (out + b2[0]).astype(np.float32)


# revision 4
# speedup vs baseline: 1.0003x; 1.0003x over previous
"""Trainium2 Bass kernel for the DependencyParser biaffine arc scorer.

scores[b,i,j] = W2 @ tanh(Wa@X[b,i] + Wb@X[b,j] + b1) + b2

Shapes (hardcoded): X [32, 96, 512], W1 [512, 1024], b1 [512],
W2 [1, 512], b2 [1].  Output [32, 96, 96] fp32.

Sharding: data-parallel over batch B=32 -> 4 batches per core x 8 cores,
weights replicated.
"""

import numpy as np
import ml_dtypes

B, N, H = 32, 96, 512
NCORES = 8
BPC = B // NCORES          # batches per core
P = 128                    # partitions
NKC = H // P               # 4 k-chunks
NHC = H // P               # 4 h-chunks
IB = 16                    # i-block size
NIB = N // IB              # 6 i-blocks per batch
FBLK = IB * N              # 1536 free elems per (kc, iblock)
NG = FBLK // 384           # 4 matmul groups of 384 cols

_CACHE = {}


def _build():
    """Build + compile the per-core Bass module (same program on all cores)."""
    import concourse.bass as bass
    import concourse.mybir as mybir
    import concourse.tile as tile
    from concourse import bacc

    f32 = mybir.dt.float32
    bf16 = mybir.dt.bfloat16
    Tanh = mybir.ActivationFunctionType.Tanh

    nc = bacc.Bacc("TRN2", target_bir_lowering=False, debug=False)

    xt_d = nc.dram_tensor("xt", [BPC, H, N], f32, kind="ExternalInput")
    wat_d = nc.dram_tensor("wat", [H, H], f32, kind="ExternalInput")
    wbt_d = nc.dram_tensor("wbt", [H, H], f32, kind="ExternalInput")
    b1_d = nc.dram_tensor("b1c", [P, NKC], f32, kind="ExternalInput")
    w2_d = nc.dram_tensor("w2c", [P, NKC], bf16, kind="ExternalInput")
    sc_d = nc.dram_tensor("scores", [BPC, N * N], f32, kind="ExternalOutput")

    with tile.TileContext(nc) as tc:
        with (
            tc.tile_pool(name="const", bufs=1) as cpool,
            tc.tile_pool(name="xt", bufs=2) as xpool,
            tc.tile_pool(name="hab", bufs=2) as habpool,
            tc.tile_pool(name="tpre", bufs=6) as tprepool,
            tc.tile_pool(name="ttan", bufs=2) as ttanpool,
            tc.tile_pool(name="scout", bufs=3) as scpool,
            tc.tile_pool(name="psum_h", bufs=2, space="PSUM") as psum_h,
            tc.tile_pool(name="psum_s", bufs=3, space="PSUM") as psum_sp,
        ):
            # ---- constants ----
            wat_s = cpool.tile([P, NHC * H], f32, tag="wat")
            wbt_s = cpool.tile([P, NHC * H], f32, tag="wbt")
            for hc in range(NHC):
                nc.sync.dma_start(
                    wat_s[:, hc * H:(hc + 1) * H], wat_d[hc * P:(hc + 1) * P, :]
                )
                nc.sync.dma_start(
                    wbt_s[:, hc * H:(hc + 1) * H], wbt_d[hc * P:(hc + 1) * P, :]
                )
            b1_s = cpool.tile([P, NKC], f32, tag="b1")
            nc.sync.dma_start(b1_s[:], b1_d[:])
            w2_s = cpool.tile([P, NKC], bf16, tag="w2")
            nc.sync.dma_start(w2_s[:], w2_d[:])

            for b in range(BPC):
                # ---- load X_b^T [512, 96] as 4 h-chunks ----
                xt_s = xpool.tile([P, NHC * N], f32)
                for hc in range(NHC):
                    nc.sync.dma_start(
                        xt_s[:, hc * N:(hc + 1) * N],
                        xt_d[b, hc * P:(hc + 1) * P, :],
                    )

                # ---- Ha/Hb = Wa@X^T, Wb@X^T  (fp32, k on partitions) ----
                ps_ha = psum_h.tile([P, NKC * N], f32, tag="ha")
                ps_hb = psum_h.tile([P, NKC * N], f32, tag="hb")
                for kc in range(NKC):
                    for hc in range(NHC):
                        nc.tensor.matmul(
                            ps_ha[:, kc * N:(kc + 1) * N],
                            wat_s[:, hc * H + kc * P: hc * H + (kc + 1) * P],
                            xt_s[:, hc * N:(hc + 1) * N],
                            start=(hc == 0),
                            stop=(hc == NHC - 1),
                        )
                for kc in range(NKC):
                    for hc in range(NHC):
                        nc.tensor.matmul(
                            ps_hb[:, kc * N:(kc + 1) * N],
                            wbt_s[:, hc * H + kc * P: hc * H + (kc + 1) * P],
                            xt_s[:, hc * N:(hc + 1) * N],
                            start=(hc == 0),
                            stop=(hc == NHC - 1),
                        )

                # ---- move to SBUF; fold b1 into Ha (stays fp32) ----
                ha_s = habpool.tile([P, NKC * N], f32, tag="ha_s")
                for kc in range(NKC):
                    nc.vector.tensor_scalar_add(
                        ha_s[:, kc * N:(kc + 1) * N],
                        ps_ha[:, kc * N:(kc + 1) * N],
                        b1_s[:, kc:kc + 1],
                    )
                hb_s = habpool.tile([P, NKC * N], bf16, tag="hb_s")
                nc.vector.tensor_copy(hb_s[:], ps_hb[:])

                # ---- main loop: broadcast-add + tanh + W2 contraction ----
                for ib in range(NIB):
                    ttan = ttanpool.tile([P, NKC * FBLK], bf16)
                    for kc in range(NKC):
                        tpre = tprepool.tile([P, FBLK], bf16)
                        for i in range(IB):
                            ii = ib * IB + i
                            nc.vector.tensor_scalar_add(
                                tpre[:, i * N:(i + 1) * N],
                                hb_s[:, kc * N:(kc + 1) * N],
                                ha_s[:, kc * N + ii: kc * N + ii + 1],
                            )
                        nc.scalar.activation(
                            ttan[:, kc * FBLK:(kc + 1) * FBLK], tpre[:], Tanh
                        )
                    ps_s = psum_sp.tile([P, 384], f32)
                    for g in range(NG):
                        for kc in range(NKC):
                            nc.tensor.matmul(
                                ps_s[32 * g:32 * g + 1, :],
                                w2_s[:, kc:kc + 1],
                                ttan[:, kc * FBLK + g * 384: kc * FBLK + (g + 1) * 384],
                                start=(kc == 0),
                                stop=(kc == NKC - 1),
                                tile_position=(0, 32 * g),
                            )
                    sc_s = scpool.tile([P, 384], f32)
                    nc.vector.tensor_copy(sc_s[:], ps_s[:])
                    sc_view = sc_s[:].rearrange("(g r) f -> g r f", r=32)[:, 0, :]
                    nc.sync.dma_start(
                        sc_d[b, ib * FBLK:(ib + 1) * FBLK].rearrange(
                            "(g f) -> g f", g=NG
                        ),
                        sc_view,
                    )

    nc.compile()
    return nc


def _get_nc():
    if "nc" not in _CACHE:
        _CACHE["nc"] = _build()
    return _CACHE["nc"]


def kernel(encoded_sequence, W1, b1, W2, b2):
    from concourse import bass_utils

    nc = _get_nc()

    x = np.asarray(encoded_sequence, dtype=np.float32)
    W1 = np.asarray(W1, dtype=np.float32)
    b1 = np.asarray(b1, dtype=np.float32)
    W2 = np.asarray(W2, dtype=np.float32)
    b2 = np.asarray(b2, dtype=np.float32)

    wat = np.ascontiguousarray(W1[:, :H].T)           # [h, k]
    wbt = np.ascontiguousarray(W1[:, H:].T)           # [h, k]
    b1c = np.ascontiguousarray(b1.reshape(NKC, P).T)  # [128, 4]
    w2c = np.ascontiguousarray(W2[0].reshape(NKC, P).T).astype(ml_dtypes.bfloat16)
    xt = np.ascontiguousarray(x.transpose(0, 2, 1))   # [B, h, n]

    in_maps = []
    for c in range(NCORES):
        in_maps.append({
            "xt": np.ascontiguousarray(xt[c * BPC:(c + 1) * BPC]),
            "wat": wat,
            "wbt": wbt,
            "b1c": b1c,
            "w2c": w2c,
        })

    res = bass_utils.run_bass_kernel_spmd(nc, in_maps, core_ids=list(range(NCORES)))
    out = np.concatenate(
        [res.results[c]["scores"].reshape(BPC, N, N) for c in range(NCORES)], axis=0
    )
    return (out + b2[0]).astype(np.float32)


# revision 6
# speedup vs baseline: 1.6259x; 1.6254x over previous
"""Trainium2 Bass kernel for the DependencyParser biaffine arc scorer.

scores[b,i,j] = W2 @ tanh(Wa@X[b,i] + Wb@X[b,j] + b1) + b2

Shapes (hardcoded): X [32, 96, 512], W1 [512, 1024], b1 [512],
W2 [1, 512], b2 [1].  Output [32, 96, 96] fp32.

Sharding: data-parallel over batch B=32 -> 4 batches per core x 8 cores,
weights replicated.

Per-core schedule:
  1. PE: Ha/Hb = Wa@X^T, Wb@X^T for all 4 local batches at once
     (moving operand packs (batch, i) -> N=384 columns), k on partitions.
  2. DVE: fold b1 into Ha (psum->sbuf, bf16); build packed bf16-pair
     copies of Ha so an int32-viewed broadcast copy (2x_2p mode)
     materializes Ha[:, i] replicated 96x along j.
  3. DVE: one big tensor_tensor add per (batch, kc, i-halfblock):
     tpre[k, (i,j)] = HaB + Hb  (bf16 2x mode).
  4. ACT: tanh over [128, 4608] tiles.
  5. PE: W2 contraction, M=1 matmuls col-tiled via tile_position so 4
     groups of 384 cols land on psum partitions 0/32/64/96.
  6. DVE copies scores psum->sbuf; DMA to DRAM.
"""

import numpy as np
import ml_dtypes

B, N, H = 32, 96, 512
NCORES = 8
BPC = B // NCORES          # batches per core
P = 128                    # partitions
NKC = H // P               # 4 k-chunks
NHC = H // P               # 4 h-chunks
NB4 = BPC * N              # 384 = batched moving cols
IB = 48                    # i-block size
NIB = N // IB              # 2 i-blocks per batch
FBLK = IB * N              # 4608 free elems per (kc, iblock)
NG = FBLK // 384           # 12 matmul groups of 384 cols per iblock

_CACHE = {}


def _build():
    """Build + compile the per-core Bass module (same program on all cores)."""
    import concourse.bass as bass
    import concourse.mybir as mybir
    import concourse.tile as tile
    from concourse import bacc

    f32 = mybir.dt.float32
    bf16 = mybir.dt.bfloat16
    i32 = mybir.dt.int32
    Tanh = mybir.ActivationFunctionType.Tanh

    nc = bacc.Bacc("TRN2", target_bir_lowering=False, debug=False)

    xt_d = nc.dram_tensor("xt", [BPC, H, N], f32, kind="ExternalInput")
    wat_d = nc.dram_tensor("wat", [H, H], f32, kind="ExternalInput")
    wbt_d = nc.dram_tensor("wbt", [H, H], f32, kind="ExternalInput")
    b1_d = nc.dram_tensor("b1c", [P, NKC], f32, kind="ExternalInput")
    w2_d = nc.dram_tensor("w2c", [P, NKC], bf16, kind="ExternalInput")
    sc_d = nc.dram_tensor("scores", [BPC, N * N], f32, kind="ExternalOutput")

    with tile.TileContext(nc) as tc:
        with (
            tc.tile_pool(name="const", bufs=1) as cpool,
            tc.tile_pool(name="t0", bufs=3) as t0pool,
            tc.tile_pool(name="t1", bufs=3) as t1pool,
            tc.tile_pool(name="ttan", bufs=2) as ttanpool,
            tc.tile_pool(name="scout", bufs=4) as scpool,
            tc.tile_pool(name="psum_h", bufs=1, space="PSUM") as psum_h,
            tc.tile_pool(name="psum_s", bufs=2, space="PSUM") as psum_sp,
        ):
            # ---- constants ----
            wat_s = cpool.tile([P, NHC * H], f32, tag="wat")
            wbt_s = cpool.tile([P, NHC * H], f32, tag="wbt")
            for hc in range(NHC):
                nc.sync.dma_start(
                    wat_s[:, hc * H:(hc + 1) * H], wat_d[hc * P:(hc + 1) * P, :]
                )
                nc.sync.dma_start(
                    wbt_s[:, hc * H:(hc + 1) * H], wbt_d[hc * P:(hc + 1) * P, :]
                )
            b1_s = cpool.tile([P, NKC], f32, tag="b1")
            nc.sync.dma_start(b1_s[:], b1_d[:])
            w2_s = cpool.tile([P, NKC], bf16, tag="w2")
            nc.sync.dma_start(w2_s[:], w2_d[:])

            # ---- load X^T for all 4 batches: cols (b, i) ----
            xt_s = cpool.tile([P, NHC * NB4], f32, tag="xt")
            for hc in range(NHC):
                for b in range(BPC):
                    nc.sync.dma_start(
                        xt_s[:, hc * NB4 + b * N: hc * NB4 + (b + 1) * N],
                        xt_d[b, hc * P:(hc + 1) * P, :],
                    )

            # ---- Ha/Hb for all batches; fold b1; pack Ha pairs ----
            ha_s = cpool.tile([P, NKC * NB4], bf16, tag="ha_s")
            hb_s = cpool.tile([P, NKC * NB4], bf16, tag="hb_s")
            happ = cpool.tile([P, NKC * NB4 * 2], bf16, tag="happ")
            for kc in range(NKC):
                ps_a = psum_h.tile([P, NB4], f32, tag="ha")
                ps_b = psum_h.tile([P, NB4], f32, tag="hb")
                for hc in range(NHC):
                    nc.tensor.matmul(
                        ps_a[:],
                        wat_s[:, hc * H + kc * P: hc * H + (kc + 1) * P],
                        xt_s[:, hc * NB4:(hc + 1) * NB4],
                        start=(hc == 0),
                        stop=(hc == NHC - 1),
                    )
                for hc in range(NHC):
                    nc.tensor.matmul(
                        ps_b[:],
                        wbt_s[:, hc * H + kc * P: hc * H + (kc + 1) * P],
                        xt_s[:, hc * NB4:(hc + 1) * NB4],
                        start=(hc == 0),
                        stop=(hc == NHC - 1),
                    )
                nc.vector.tensor_scalar_add(
                    ha_s[:, kc * NB4:(kc + 1) * NB4], ps_a[:], b1_s[:, kc:kc + 1]
                )
                nc.vector.tensor_copy(hb_s[:, kc * NB4:(kc + 1) * NB4], ps_b[:])
                # duplicate each Ha value into adjacent bf16 pairs
                hpv = happ[:, kc * NB4 * 2:(kc + 1) * NB4 * 2].rearrange(
                    "p (i two) -> p i two", two=2
                )
                nc.vector.tensor_copy(
                    hpv[:, :, 0], ha_s[:, kc * NB4:(kc + 1) * NB4]
                )
                nc.vector.tensor_copy(
                    hpv[:, :, 1], ha_s[:, kc * NB4:(kc + 1) * NB4]
                )

            # ---- main loop ----
            for b in range(BPC):
                for ib in range(NIB):
                    i0 = b * N + ib * IB  # column offset into (b, i) packing
                    ttan = ttanpool.tile([P, NKC * FBLK], bf16)
                    for kc in range(NKC):
                        # broadcast Ha[:, i] along j via int32 pair copy
                        t0 = t0pool.tile([P, FBLK], bf16)
                        src32 = happ[:, (kc * NB4 + i0) * 2:
                                     (kc * NB4 + i0 + IB) * 2].bitcast(i32)
                        b32 = src32.unsqueeze(2).broadcast_to([P, IB, N // 2])
                        d32 = t0[:].bitcast(i32).rearrange(
                            "p (i j) -> p i j", i=IB
                        )
                        nc.vector.tensor_copy(d32, b32)
                        # + Hb  (bf16 2x: inner dims step-1 on all operands)
                        t1 = t1pool.tile([P, FBLK], bf16)
                        hbv = hb_s[:, kc * NB4 + b * N: kc * NB4 + (b + 1) * N]
                        hb3 = hbv.unsqueeze(1).broadcast_to([P, IB, N])
                        t13 = t1[:].rearrange("p (i j) -> p i j", i=IB)
                        t03 = t0[:].rearrange("p (i j) -> p i j", i=IB)
                        nc.vector.tensor_add(t13, t03, hb3)
                        nc.scalar.activation(
                            ttan[:, kc * FBLK:(kc + 1) * FBLK], t1[:], Tanh
                        )
                    for t in range(NG // 4):
                        ps_s = psum_sp.tile([P, 384], f32, tag=f"s{t}")
                        for gg in range(4):
                            g = t * 4 + gg
                            for kc in range(NKC):
                                nc.tensor.matmul(
                                    ps_s[32 * gg:32 * gg + 1, :],
                                    w2_s[:, kc:kc + 1],
                                    ttan[:, kc * FBLK + g * 384:
                                         kc * FBLK + (g + 1) * 384],
                                    start=(kc == 0),
                                    stop=(kc == NKC - 1),
                                    tile_position=(0, 32 * gg),
                                )
                        sc_s = scpool.tile([P, 384], f32)
                        nc.vector.tensor_copy(sc_s[:], ps_s[:])
                        sc_view = sc_s[:].rearrange("(g r) f -> g r f", r=32)[:, 0, :]
                        nc.sync.dma_start(
                            sc_d[b, ib * FBLK + t * 1536:
                                 ib * FBLK + (t + 1) * 1536].rearrange(
                                "(g f) -> g f", g=4
                            ),
                            sc_view,
                        )

    nc.compile()
    return nc


def _get_nc():
    if "nc" not in _CACHE:
        _CACHE["nc"] = _build()
    return _CACHE["nc"]


def _make_in_maps(encoded_sequence, W1, b1, W2):
    x = np.asarray(encoded_sequence, dtype=np.float32)
    W1 = np.asarray(W1, dtype=np.float32)
    b1 = np.asarray(b1, dtype=np.float32)
    W2 = np.asarray(W2, dtype=np.float32)

    wat = np.ascontiguousarray(W1[:, :H].T)           # [h, k]
    wbt = np.ascontiguousarray(W1[:, H:].T)           # [h, k]
    b1c = np.ascontiguousarray(b1.reshape(NKC, P).T)  # [128, 4]
    w2c = np.ascontiguousarray(W2[0].reshape(NKC, P).T).astype(ml_dtypes.bfloat16)
    xt = np.ascontiguousarray(x.transpose(0, 2, 1))   # [B, h, n]

    in_maps = []
    for c in range(NCORES):
        in_maps.append({
            "xt": np.ascontiguousarray(xt[c * BPC:(c + 1) * BPC]),
            "wat": wat,
            "wbt": wbt,
            "b1c": b1c,
            "w2c": w2c,
        })
    return in_maps


def kernel(encoded_sequence, W1, b1, W2, b2):
    from concourse import bass_utils

    nc = _get_nc()
    in_maps = _make_in_maps(encoded_sequence, W1, b1, W2)
    res = bass_utils.run_bass_kernel_spmd(nc, in_maps, core_ids=list(range(NCORES)))
    out = np.concatenate(
        [res.results[c]["scores"].reshape(BPC, N, N) for c in range(NCORES)], axis=0
    )
    b2 = np.asarray(b2, dtype=np.float32)
    return (out + b2[0]).astype(np.float32)
